# revision 28
# baseline (speedup 1.0000x reference)
"""Multi-head attention (B=2, S=2048, D=1024, H=16) on 8 Trainium2 NeuronCores.

Sharding (per the batch+head hint): core c handles batch b=c//4 and head-group
g=c%4 (4 heads, i.e. a 256-column slice of the QKV projections and a 256-row
slice of Wo).  Each core computes q^T/k^T/v projections for its head group,
flash-style attention in transposed score space (scores^T = k^T-tile.T @ q^T,
softmax denominator via a ones-augmented V column in the PV matmul), and its
out-projection partial  ctx_g @ Wo[256g:256(g+1), :].

The out_proj reduction over the 4 head-group cores of each batch is done on
the host (device collectives on this stack cost ~145us for 8MB - far more
than the arithmetic they replace - so the partial-sum gather IS the unshard
step).  Biases: bq/bk are applied on device (they feed the softmax
nonlinearly); bv/bo commute through attention/out_proj linearly and are folded
into a single host-side correction vector  c = bv @ Wo + bo.

Performance structure (v2): the PE array is the binding engine, so emission
is software-pipelined to keep it saturated:
 - all matmul operands are bf16 (1 col/cycle streaming; LDWEIGHTS pipelines
   behind the previous matmul, so back-to-back cadence is ~N/2.4GHz).
 - PV matmuls lag the scores/exp stream by a few iterations so the PE never
   waits on the Activation engine's exp.
 - out-projection of chunk j-1 and q-projection of chunk j+1 are sprinkled
   into chunk j's attention loop to fill residual exp-wait gaps.
 - softmax normalization is fully on-chip: for odd heads the V block is
   stored [ones|v] and the PV output lands at psum partitions 63..127, so
   ctx rows 64..127 are written in place (no partition-shift DMA).  The
   denominator reciprocal row is broadcast across partitions with a rank-1
   matmul (ones ⊗ recip) instead of a DRAM round trip.
 - PSUM->SBUF copies ride the otherwise-idle GpSimd engine.
"""

import numpy as np
import ml_dtypes

import concourse.bass as bass
import concourse.mybir as mybir
import concourse.tile as tile
from concourse import bacc
from concourse.bass_utils import run_bass_kernel_spmd

B, S, D, H = 2, 2048, 1024, 16
HD = D // H          # 64 head dim
NCORE = 8
G = NCORE // B       # 4 head-groups per batch
HG = H // G          # 4 heads per group
DG = D // G          # 256 projection columns per group
P = 128              # partitions
KT = D // P          # 8 contraction tiles for projections
CH = 512             # s-chunk (projection rhs width & attention sq chunk)
NJ = S // CH         # 4 chunks
STILES = S // P      # 16 sk tiles
TB = 2               # scores t-batch per exp op (psum: [128, TB*512] = 2 banks)
VBLK = HD + 1        # v block: 64 v cols + 1 ones col (softmax denominator)
LAG = 3              # PV emission lags scores/exp by this many tb-iterations

f32 = mybir.dt.float32
bf16 = mybir.dt.bfloat16
MM_DT = mybir.dt.bfloat16
NP_MM = np.float32 if MM_DT == mybir.dt.float32r else ml_dtypes.bfloat16
EXP = mybir.ActivationFunctionType.Exp
SCALE = 1.0 / np.sqrt(np.float32(HD))


def _build_program():
    nc = bacc.Bacc("TRN2", target_bir_lowering=False, debug=False,
                   num_devices=NCORE)

    xqT_d = nc.dram_tensor("xqT", [D, S], MM_DT, kind="ExternalInput")
    xkT_d = nc.dram_tensor("xkT", [D, S], MM_DT, kind="ExternalInput")
    xvT_d = nc.dram_tensor("xvT", [D, S], MM_DT, kind="ExternalInput")
    wq_d = nc.dram_tensor("wq", [D, DG], MM_DT, kind="ExternalInput")
    wk_d = nc.dram_tensor("wk", [D, DG], MM_DT, kind="ExternalInput")
    wv_d = nc.dram_tensor("wv", [D, DG], MM_DT, kind="ExternalInput")
    wo_d = nc.dram_tensor("wo", [DG, D], MM_DT, kind="ExternalInput")
    bq_d = nc.dram_tensor("bqk", [2, DG], f32, kind="ExternalInput")
    out_d = nc.dram_tensor("out", [S, D], f32, kind="ExternalOutput")

    with tile.TileContext(nc) as tc:
        _emit(nc, tc, xqT_d, xkT_d, xvT_d, wq_d, wk_d, wv_d, wo_d, bq_d, out_d)
    nc.compile()
    return nc


def _emit(nc, tc, xqT_d, xkT_d, xvT_d, wq_d, wk_d, wv_d, wo_d, bq_d, out_d):
    from collections import deque
    from contextlib import ExitStack
    ctx = ExitStack()
    with ctx:
        consts = ctx.enter_context(tc.tile_pool(name="consts", bufs=1))
        persist = ctx.enter_context(tc.tile_pool(name="persist", bufs=1))
        xpool = ctx.enter_context(tc.tile_pool(name="xchunk", bufs=6))
        epool = ctx.enter_context(tc.tile_pool(name="exps", bufs=10))
        small = ctx.enter_context(tc.tile_pool(name="small", bufs=8))
        bpool = ctx.enter_context(tc.tile_pool(name="bcast", bufs=4))
        opool = ctx.enter_context(tc.tile_pool(name="ostage", bufs=3))
        drbp = ctx.enter_context(tc.tile_pool(name="drb", bufs=4, space="DRAM"))
        ps_s = ctx.enter_context(tc.tile_pool(name="ps_s", bufs=2, space="PSUM"))
        ps_acc = ctx.enter_context(tc.tile_pool(name="ps_acc", bufs=2, space="PSUM"))
        ps_x = ctx.enter_context(tc.tile_pool(name="ps_x", bufs=2, space="PSUM"))

        def load_xchunk(x_d, j, eng=None):
            # spread input streams across DGE queues so transfers overlap
            t = xpool.tile([P, KT * CH], MM_DT, tag="xchunk", name="xchunk")
            (eng or nc.sync).dma_start(
                out=t.rearrange("p (kt s) -> p kt s", kt=KT),
                in_=x_d[:, j * CH:(j + 1) * CH].rearrange(
                    "(kt p) s -> p kt s", p=P))
            return t.rearrange("p (kt s) -> p kt s", kt=KT)

        # ---- constants: wk first (k-projection starts the kernel) -----
        wk_sb = consts.tile([P, KT * DG], MM_DT, tag="wk")
        nc.sync.dma_start(out=wk_sb.rearrange("p (kt m) -> p kt m", kt=KT),
                          in_=wk_d.rearrange("(kt p) m -> p kt m", p=P))
        xc0 = load_xchunk(xkT_d, 0)

        wq_sb = consts.tile([P, KT * DG], MM_DT, tag="wq")
        wv_sb = consts.tile([P, KT * DG], MM_DT, tag="wv")
        for w_sb, w_d in ((wv_sb, wv_d), (wq_sb, wq_d)):
            nc.sync.dma_start(
                out=w_sb.rearrange("p (kt m) -> p kt m", kt=KT),
                in_=w_d.rearrange("(kt p) m -> p kt m", p=P))
        wo_sb = consts.tile([P, 2 * D], MM_DT, tag="wo")  # 2 k-tiles [128, D]
        nc.sync.dma_start(out=wo_sb.rearrange("p (kt n) -> p kt n", kt=2),
                          in_=wo_d.rearrange("(kt p) n -> p kt n", p=P))
        bqk_sb = consts.tile([P, 4], f32, tag="bqk")  # [bq|bk] x m-half
        nc.sync.dma_start(out=bqk_sb.rearrange("p (i mh) -> p i mh", i=2),
                          in_=bq_d.rearrange("i (mh p) -> p i mh", p=P))
        # persistent activations
        qT = [persist.tile([P, S], MM_DT, tag=f"qT{m}", name=f"qT{m}")
              for m in range(2)]
        kT = [persist.tile([P, S], MM_DT, tag=f"kT{m}", name=f"kT{m}")
              for m in range(2)]
        v_sb = persist.tile([P, HG * STILES * VBLK], MM_DT, tag="v")
        ctxT = [persist.tile([P, S], MM_DT, tag=f"ctxT{m}", name=f"ctxT{m}")
                for m in range(2)]
        v_view = v_sb.rearrange("p (h t c) -> p h t c", h=HG, t=STILES)

        # ones columns for the softmax-denominator rows of the PV matmuls
        ones_d = nc.inline_tensor(
            np.ones((P, HG * STILES), NP_MM), name="ones_col")
        nc.sync.dma_start(out=v_view[:, :, :, HD],
                          in_=ones_d.ap().rearrange("p (h t) -> p h t", h=HG))

        def proj_qk(xc, w_sb, dst, bias_i, j):
            # dst[m][dq, j*CH:+CH] = (W[:, m-half].T @ x^T-chunk) + bias
            for m in range(2):
                acc = ps_x.tile([P, CH], f32, tag="px", name="px")
                for k in range(KT):
                    nc.tensor.matmul(
                        acc[:], w_sb[:, k * DG + m * P:k * DG + (m + 1) * P],
                        xc[:, k, :], start=(k == 0), stop=(k == KT - 1))
                nc.vector.tensor_add(
                    dst[m][:, j * CH:(j + 1) * CH], acc[:],
                    bqk_sb[:, 2 * bias_i + m:2 * bias_i + m + 1].broadcast_to(
                        [P, CH]))

        def proj_v(xc, j):
            # v rows j*CH..: 4 s-subtiles of 128; heads land in v_view blocks
            for si in range(CH // P):
                st = j * (CH // P) + si
                acc = ps_x.tile([P, DG], f32, tag="px", name="px")
                for k in range(KT):
                    nc.tensor.matmul(
                        acc[:], xc[:, k, si * P:(si + 1) * P],
                        wv_sb[:, k * DG:(k + 1) * DG],
                        start=(k == 0), stop=(k == KT - 1))
                nc.scalar.activation(
                    v_view[:, :, st, 0:HD],
                    acc[:].rearrange("p (h c) -> p h c", h=HG),
                    mybir.ActivationFunctionType.Copy)

        # ---- head phase: k (full S), q chunk 0, v (full S) -------------
        # xq/xv ride the Activation/GpSimd DGE queues so their transfers
        # overlap the xk stream on the SP queue.
        xq0 = load_xchunk(xqT_d, 0, eng=nc.scalar)
        xvc = [load_xchunk(xvT_d, j, eng=nc.gpsimd) for j in range(2)]
        proj_qk(xc0, wk_sb, kT, 1, 0)
        for j in range(1, NJ):
            xc = load_xchunk(xkT_d, j)
            proj_qk(xc, wk_sb, kT, 1, j)
        proj_qk(xq0, wq_sb, qT, 0, 0)
        for j in range(NJ):
            proj_v(xvc[j], j)
            if j + 2 < NJ:
                xvc.append(load_xchunk(xvT_d, j + 2, eng=nc.gpsimd))

        # ---- deferred-work queues --------------------------------------
        # pvq: ordered PV/normalize closures (softmax-side pipeline, popped
        #      1 per tb-iteration, lagging production by LAG iterations).
        # sprinkle: independent PE work (out-proj of j-1, q-proj of j+1)
        #      popped up to 2 units per iteration to fill exp-wait gaps.
        pvq = deque()
        sprinkle = deque()

        def pop_pv():
            if len(pvq) > LAG:
                fn = pvq.popleft()
                again = fn()
                if again and len(pvq) > LAG:  # normalize is cheap: pop 2
                    pvq.popleft()()

        def pop_sprinkle(n=2):
            for _ in range(min(n, len(sprinkle))):
                sprinkle.popleft()()

        def emit_outproj_unit(jj, si, nh):
            def fn():
                st = jj * (CH // P) + si
                po = ps_x.tile([P, CH], f32, tag="px", name="px")
                for m in range(2):
                    nc.tensor.matmul(
                        po[:], ctxT[m][:, st * P:(st + 1) * P],
                        wo_sb[:, m * D + nh * CH:m * D + (nh + 1) * CH],
                        start=(m == 0), stop=(m == 1))
                ostage = opool.tile([P, CH], f32, tag="ostage", name="ostage")
                nc.vector.tensor_copy(ostage[:], po[:])
                nc.sync.dma_start(
                    out_d[st * P:(st + 1) * P, nh * CH:(nh + 1) * CH],
                    ostage[:])
            return fn

        def emit_qproj_unit(xc, m, jj):
            def fn():
                acc = ps_x.tile([P, CH], f32, tag="px", name="px")
                for k in range(KT):
                    nc.tensor.matmul(
                        acc[:], wq_sb[:, k * DG + m * P:k * DG + (m + 1) * P],
                        xc[:, k, :], start=(k == 0), stop=(k == KT - 1))
                nc.vector.tensor_add(
                    qT[m][:, jj * CH:(jj + 1) * CH], acc[:],
                    bqk_sb[:, m:m + 1].broadcast_to([P, CH]))
            return fn

        # ---- attention: global software-pipelined iteration stream -----
        for j in range(NJ):
            jc = slice(j * CH, (j + 1) * CH)
            if j + 1 < NJ:
                xq_next = load_xchunk(xqT_d, j + 1)
                xq_stage = xq_next

            for m in range(2):
                # q-proj of j+1 staged at m=1 so its input DMA has landed
                if m == 1 and j + 1 < NJ:
                    for mm_ in range(2):
                        sprinkle.append(emit_qproj_unit(xq_stage, mm_, j + 1))
                # psum accumulators for this head pair (ctx 0-63, den 64)
                accs = [ps_acc.tile([P, CH], f32, tag="acc", name=f"acc{i}")
                        for i in range(2)]
                avs = [accs[0][0:VBLK, :], accs[1][0:VBLK, :]]

                for t in range(STILES):
                    # both heads' scores for sk-tile t in ONE psum tile so
                    # the pool's 2 bufs give true double-buffering, and one
                    # exp instruction covers both heads
                    sps = ps_s.tile([P, 2 * CH], f32, tag="s", name="s")
                    for hh in range(2):
                        lo, hi = hh * HD, (hh + 1) * HD
                        nc.tensor.matmul(
                            sps[:, hh * CH:(hh + 1) * CH],
                            kT[m][lo:hi, t * P:(t + 1) * P],
                            qT[m][lo:hi, jc], start=True, stop=True)
                    e = epool.tile([P, 2 * CH], MM_DT, tag="e", name="e")
                    nc.scalar.activation(e[:], sps[:], EXP, scale=SCALE)

                    def emit_pv(m=m, t=t, e=e, avs=avs):
                        for hh in range(2):
                            nc.tensor.matmul(
                                avs[hh], v_view[:, 2 * m + hh, t, :],
                                e[:, hh * CH:(hh + 1) * CH],
                                start=(t == 0), stop=(t == STILES - 1))
                        return False
                    pvq.append(emit_pv)
                    pop_pv()
                    pop_sprinkle()

                def emit_norm(m=m, j=j, jc=jc, accs=accs):
                    # normalize ctx by the softmax denominators (psum row 64
                    # of each acc).  DVE reciprocal cost scales with per-lane
                    # free size, so reshape the two [1,512] rows to [128,8]
                    # (sbuf->sbuf DMA crosses partitions), recip there, then
                    # bounce through DRAM for a stride-0 partition broadcast
                    # back to [64,512].  The whole chain runs on DVE/DMA and
                    # is deferred, so the PE never waits on it.
                    rs = small.tile([1, 2 * CH], f32, tag="rs", name="rs")
                    for hh in range(2):
                        nc.vector.tensor_copy(rs[0:1, hh * CH:(hh + 1) * CH],
                                              accs[hh][HD:HD + 1, :])
                    nq = CH // P
                    rq = small.tile([P, 2 * nq], f32, tag="rq", name="rq")
                    for hh in range(2):
                        nc.gpsimd.dma_start(
                            rq[:, hh * nq:(hh + 1) * nq],
                            rs[0:1, hh * CH:(hh + 1) * CH].rearrange(
                                "o (p c) -> o p c", p=P))
                    rr = small.tile([P, 2 * nq], f32, tag="rr", name="rr")
                    nc.vector.reciprocal(rr[:], rq[:])
                    drb = drbp.tile([2, CH], f32, tag="drb", name="drb")
                    for hh in range(2):
                        nc.gpsimd.dma_start(
                            drb[hh:hh + 1, :].rearrange(
                                "o (p c) -> (o p) c", p=P),
                            rr[:, hh * nq:(hh + 1) * nq])
                    bbs = []
                    for hh in range(2):
                        bb = bpool.tile([HD, CH], f32, tag="bb", name="bb")
                        nc.gpsimd.dma_start(
                            bb[:], drb[hh:hh + 1, :].broadcast_to([HD, CH]))
                        bbs.append(bb)
                    nc.vector.tensor_mul(ctxT[m][0:HD, jc],
                                         accs[0][0:HD, :], bbs[0][:])
                    tmp = small.tile([HD, CH], MM_DT, tag="tmp", name="tmp")
                    nc.vector.tensor_mul(tmp[:], accs[1][0:HD, :], bbs[1][:])
                    nc.gpsimd.dma_start(ctxT[m][HD:P, jc], tmp[:])
                    if m == 1:
                        # ctxT for chunk j complete: its out-proj may now be
                        # scheduled (pops during the next chunk's loop)
                        for si in range(CH // P):
                            for nh in range(2):
                                sprinkle.append(emit_outproj_unit(j, si, nh))
                    return True
                pvq.append(emit_norm)

        # ---- drain: remaining PV/normalize + final chunk out-proj ------
        while pvq:
            pvq.popleft()()
            pop_sprinkle()
        while sprinkle:
            sprinkle.popleft()()


_NC_CACHE = {}


def _get_program():
    if "nc" not in _NC_CACHE:
        _NC_CACHE["nc"] = _build_program()
    return _NC_CACHE["nc"]


def _make_in_maps(inputs):
    query = np.asarray(inputs["query"], dtype=np.float32)
    key = np.asarray(inputs["key"], dtype=np.float32)
    value = np.asarray(inputs["value"], dtype=np.float32)
    Wq = np.asarray(inputs["Wq"], dtype=np.float32)
    Wk = np.asarray(inputs["Wk"], dtype=np.float32)
    Wv = np.asarray(inputs["Wv"], dtype=np.float32)
    Wo = np.asarray(inputs["Wo"], dtype=np.float32)
    bq = np.asarray(inputs["bq"], dtype=np.float32)
    bk = np.asarray(inputs["bk"], dtype=np.float32)

    xT = {}
    for b in range(B):
        xT[("q", b)] = np.ascontiguousarray(query[b].T).astype(NP_MM)
        xT[("k", b)] = np.ascontiguousarray(key[b].T).astype(NP_MM)
        xT[("v", b)] = np.ascontiguousarray(value[b].T).astype(NP_MM)

    in_maps = []
    for c in range(NCORE):
        b, g = divmod(c, G)
        cols = slice(g * DG, (g + 1) * DG)
        in_maps.append({
            "xqT": xT[("q", b)],
            "xkT": xT[("k", b)],
            "xvT": xT[("v", b)],
            "wq": np.ascontiguousarray(Wq[:, cols]).astype(NP_MM),
            "wk": np.ascontiguousarray(Wk[:, cols]).astype(NP_MM),
            "wv": np.ascontiguousarray(Wv[:, cols]).astype(NP_MM),
            "wo": np.ascontiguousarray(Wo[cols, :]).astype(NP_MM),
            "bqk": np.ascontiguousarray(np.stack([bq[cols], bk[cols]])),
        })
    return in_maps


def kernel(query, key, value, Wq, bq, Wk, bk, Wv, bv, Wo, bo):
    bv = np.asarray(bv, dtype=np.float32)
    bo = np.asarray(bo, dtype=np.float32)
    Wo = np.asarray(Wo, dtype=np.float32)

    nc = _get_program()
    in_maps = _make_in_maps({
        "query": query, "key": key, "value": value, "Wq": Wq, "Wk": Wk,
        "Wv": Wv, "Wo": Wo, "bq": bq, "bk": bk,
    })

    res = run_bass_kernel_spmd(nc, in_maps, list(range(NCORE)))

    # unshard: sum the 4 head-group partials per batch; add the linear bias
    # correction (bv and bo commute through attention/out_proj).
    corr = bv @ Wo + bo
    out = np.empty((B, S, D), dtype=np.float32)
    for b in range(B):
        acc = res.results[4 * b]["out"].copy()
        for g in range(1, G):
            acc += res.results[4 * b + g]["out"]
        out[b] = acc + corr
    return out


# revision 29
# speedup vs baseline: 1.1755x; 1.1755x over previous
"""Multi-head attention (B=2, S=2048, D=1024, H=16) on 8 Trainium2 NeuronCores.

Sharding (per the batch+head hint): core c handles batch b=c//4 and head-group
g=c%4 (4 heads, i.e. a 256-column slice of the QKV projections and a 256-row
slice of Wo).  Each core computes q^T/k^T/v projections for its head group,
flash-style attention in transposed score space (scores^T = k^T-tile.T @ q^T,
softmax denominator via a ones-augmented V column in the PV matmul), and its
out-projection partial  ctx_g @ Wo[256g:256(g+1), :].

The out_proj reduction over the 4 head-group cores of each batch is done on
the host (device collectives on this stack cost ~145us for 8MB - far more
than the arithmetic they replace - so the partial-sum gather IS the unshard
step).  Biases: bq/bk are applied on device (they feed the softmax
nonlinearly); bv/bo commute through attention/out_proj linearly and are folded
into a single host-side correction vector  c = bv @ Wo + bo.

Performance structure (v2): the PE array is the binding engine, so emission
is software-pipelined to keep it saturated:
 - all matmul operands are bf16 (1 col/cycle streaming; LDWEIGHTS pipelines
   behind the previous matmul, so back-to-back cadence is ~N/2.4GHz).
 - PV matmuls lag the scores/exp stream by a few iterations so the PE never
   waits on the Activation engine's exp.
 - out-projection of chunk j-1 and q-projection of chunk j+1 are sprinkled
   into chunk j's attention loop to fill residual exp-wait gaps.
 - softmax normalization is fully on-chip: for odd heads the V block is
   stored [ones|v] and the PV output lands at psum partitions 63..127, so
   ctx rows 64..127 are written in place (no partition-shift DMA).  The
   denominator reciprocal row is broadcast across partitions with a rank-1
   matmul (ones ⊗ recip) instead of a DRAM round trip.
 - PSUM->SBUF copies ride the otherwise-idle GpSimd engine.
"""

import numpy as np
import ml_dtypes

import concourse.bass as bass
import concourse.mybir as mybir
import concourse.tile as tile
from concourse import bacc
from concourse.bass_utils import run_bass_kernel_spmd

B, S, D, H = 2, 2048, 1024, 16
HD = D // H          # 64 head dim
NCORE = 8
G = NCORE // B       # 4 head-groups per batch
HG = H // G          # 4 heads per group
DG = D // G          # 256 projection columns per group
P = 128              # partitions
KT = D // P          # 8 contraction tiles for projections
CH = 512             # s-chunk (projection rhs width & attention sq chunk)
NJ = S // CH         # 4 chunks
STILES = S // P      # 16 sk tiles
TB = 2               # scores t-batch per exp op (psum: [128, TB*512] = 2 banks)
VBLK = HD + 1        # v block: 64 v cols + 1 ones col (softmax denominator)
LAG = 3              # PV emission lags scores/exp by this many tb-iterations

f32 = mybir.dt.float32
bf16 = mybir.dt.bfloat16
MM_DT = mybir.dt.bfloat16
NP_MM = np.float32 if MM_DT == mybir.dt.float32r else ml_dtypes.bfloat16
EXP = mybir.ActivationFunctionType.Exp
SCALE = 1.0 / np.sqrt(np.float32(HD))


def _build_program():
    nc = bacc.Bacc("TRN2", target_bir_lowering=False, debug=False,
                   num_devices=NCORE)

    # inputs/weights are pre-laid-out on the host so every DMA reads
    # contiguous 4-8KB runs per partition (strided 1KB-run descriptors only
    # reach ~100GB/s on these queues)
    xqT_d = nc.dram_tensor("xqT", [P, NJ * KT * CH], MM_DT, kind="ExternalInput")
    xkT_d = nc.dram_tensor("xkT", [P, NJ * KT * CH], MM_DT, kind="ExternalInput")
    xvT_d = nc.dram_tensor("xvT", [P, NJ * KT * CH], MM_DT, kind="ExternalInput")
    wq_d = nc.dram_tensor("wq", [P, KT * DG], MM_DT, kind="ExternalInput")
    wk_d = nc.dram_tensor("wk", [P, KT * DG], MM_DT, kind="ExternalInput")
    wv_d = nc.dram_tensor("wv", [P, KT * DG], MM_DT, kind="ExternalInput")
    wo_d = nc.dram_tensor("wo", [P, 2 * D], MM_DT, kind="ExternalInput")
    bq_d = nc.dram_tensor("bqk", [2, DG], f32, kind="ExternalInput")
    out_d = nc.dram_tensor("out", [S, D], f32, kind="ExternalOutput")

    with tile.TileContext(nc) as tc:
        _emit(nc, tc, xqT_d, xkT_d, xvT_d, wq_d, wk_d, wv_d, wo_d, bq_d, out_d)
    nc.compile()
    return nc


def _emit(nc, tc, xqT_d, xkT_d, xvT_d, wq_d, wk_d, wv_d, wo_d, bq_d, out_d):
    from collections import deque
    from contextlib import ExitStack
    ctx = ExitStack()
    with ctx:
        consts = ctx.enter_context(tc.tile_pool(name="consts", bufs=1))
        persist = ctx.enter_context(tc.tile_pool(name="persist", bufs=1))
        xpool = ctx.enter_context(tc.tile_pool(name="xchunk", bufs=6))
        epool = ctx.enter_context(tc.tile_pool(name="exps", bufs=10))
        small = ctx.enter_context(tc.tile_pool(name="small", bufs=8))
        bpool = ctx.enter_context(tc.tile_pool(name="bcast", bufs=4))
        opool = ctx.enter_context(tc.tile_pool(name="ostage", bufs=3))
        drbp = ctx.enter_context(tc.tile_pool(name="drb", bufs=4, space="DRAM"))
        ps_s = ctx.enter_context(tc.tile_pool(name="ps_s", bufs=2, space="PSUM"))
        ps_acc = ctx.enter_context(tc.tile_pool(name="ps_acc", bufs=2, space="PSUM"))
        ps_x = ctx.enter_context(tc.tile_pool(name="ps_x", bufs=2, space="PSUM"))

        def load_xchunk(x_d, j, eng=None):
            # spread input streams across DGE queues so transfers overlap
            t = xpool.tile([P, KT * CH], MM_DT, tag="xchunk", name="xchunk")
            (eng or nc.sync).dma_start(
                out=t[:],
                in_=x_d.rearrange("p (j x) -> p j x", j=NJ)[:, j])
            return t.rearrange("p (kt s) -> p kt s", kt=KT)

        # ---- constants: wk first (k-projection starts the kernel) -----
        wk_sb = consts.tile([P, KT * DG], MM_DT, tag="wk")
        nc.sync.dma_start(out=wk_sb[:], in_=wk_d.ap())
        xc0 = load_xchunk(xkT_d, 0)

        wq_sb = consts.tile([P, KT * DG], MM_DT, tag="wq")
        wv_sb = consts.tile([P, KT * DG], MM_DT, tag="wv")
        nc.scalar.dma_start(out=wq_sb[:], in_=wq_d.ap())
        nc.gpsimd.dma_start(out=wv_sb[:], in_=wv_d.ap())
        wo_sb = consts.tile([P, 2 * D], MM_DT, tag="wo")  # 2 k-tiles [128, D]
        nc.sync.dma_start(out=wo_sb[:], in_=wo_d.ap())
        bqk_sb = consts.tile([P, 4], f32, tag="bqk")  # [bq|bk] x m-half
        nc.sync.dma_start(out=bqk_sb.rearrange("p (i mh) -> p i mh", i=2),
                          in_=bq_d.rearrange("i (mh p) -> p i mh", p=P))
        # persistent activations
        qT = [persist.tile([P, S], MM_DT, tag=f"qT{m}", name=f"qT{m}")
              for m in range(2)]
        kT = [persist.tile([P, S], MM_DT, tag=f"kT{m}", name=f"kT{m}")
              for m in range(2)]
        v_sb = persist.tile([P, HG * STILES * VBLK], MM_DT, tag="v")
        ctxT = [persist.tile([P, S], MM_DT, tag=f"ctxT{m}", name=f"ctxT{m}")
                for m in range(2)]
        v_view = v_sb.rearrange("p (h t c) -> p h t c", h=HG, t=STILES)

        # ones columns for the softmax-denominator rows of the PV matmuls
        ones_d = nc.inline_tensor(
            np.ones((P, HG * STILES), NP_MM), name="ones_col")
        nc.sync.dma_start(out=v_view[:, :, :, HD],
                          in_=ones_d.ap().rearrange("p (h t) -> p h t", h=HG))

        def proj_qk(xc, w_sb, dst, bias_i, j):
            # dst[m][dq, j*CH:+CH] = (W[:, m-half].T @ x^T-chunk) + bias
            for m in range(2):
                acc = ps_x.tile([P, CH], f32, tag="px", name="px")
                for k in range(KT):
                    nc.tensor.matmul(
                        acc[:], w_sb[:, k * DG + m * P:k * DG + (m + 1) * P],
                        xc[:, k, :], start=(k == 0), stop=(k == KT - 1))
                nc.vector.tensor_add(
                    dst[m][:, j * CH:(j + 1) * CH], acc[:],
                    bqk_sb[:, 2 * bias_i + m:2 * bias_i + m + 1].broadcast_to(
                        [P, CH]))

        def proj_v(xc, j):
            # v rows j*CH..: 4 s-subtiles of 128; heads land in v_view blocks
            for si in range(CH // P):
                st = j * (CH // P) + si
                acc = ps_x.tile([P, DG], f32, tag="px", name="px")
                for k in range(KT):
                    nc.tensor.matmul(
                        acc[:], xc[:, k, si * P:(si + 1) * P],
                        wv_sb[:, k * DG:(k + 1) * DG],
                        start=(k == 0), stop=(k == KT - 1))
                nc.scalar.activation(
                    v_view[:, :, st, 0:HD],
                    acc[:].rearrange("p (h c) -> p h c", h=HG),
                    mybir.ActivationFunctionType.Copy)

        # ---- head phase: k (full S), q chunk 0, v (full S) -------------
        # xq/xv ride the Activation/GpSimd DGE queues so their transfers
        # overlap the xk stream on the SP queue.
        xq0 = load_xchunk(xqT_d, 0, eng=nc.scalar)
        xvc = [load_xchunk(xvT_d, j, eng=nc.gpsimd) for j in range(2)]
        proj_qk(xc0, wk_sb, kT, 1, 0)
        for j in range(1, NJ):
            xc = load_xchunk(xkT_d, j)
            proj_qk(xc, wk_sb, kT, 1, j)
        proj_qk(xq0, wq_sb, qT, 0, 0)
        for j in range(NJ):
            proj_v(xvc[j], j)
            if j + 2 < NJ:
                xvc.append(load_xchunk(xvT_d, j + 2, eng=nc.gpsimd))

        # ---- deferred-work queues --------------------------------------
        # pvq: ordered PV/normalize closures (softmax-side pipeline, popped
        #      1 per tb-iteration, lagging production by LAG iterations).
        # sprinkle: independent PE work (out-proj of j-1, q-proj of j+1)
        #      popped up to 2 units per iteration to fill exp-wait gaps.
        pvq = deque()
        sprinkle = deque()

        def pop_pv():
            if len(pvq) > LAG:
                fn = pvq.popleft()
                again = fn()
                if again and len(pvq) > LAG:  # normalize is cheap: pop 2
                    pvq.popleft()()

        def pop_sprinkle(n=2):
            for _ in range(min(n, len(sprinkle))):
                sprinkle.popleft()()

        def emit_outproj_unit(jj, si, nh):
            def fn():
                st = jj * (CH // P) + si
                po = ps_x.tile([P, CH], f32, tag="px", name="px")
                for m in range(2):
                    nc.tensor.matmul(
                        po[:], ctxT[m][:, st * P:(st + 1) * P],
                        wo_sb[:, m * D + nh * CH:m * D + (nh + 1) * CH],
                        start=(m == 0), stop=(m == 1))
                ostage = opool.tile([P, CH], f32, tag="ostage", name="ostage")
                nc.vector.tensor_copy(ostage[:], po[:])
                nc.sync.dma_start(
                    out_d[st * P:(st + 1) * P, nh * CH:(nh + 1) * CH],
                    ostage[:])
            return fn

        def emit_qproj_unit(xc, m, jj):
            def fn():
                acc = ps_x.tile([P, CH], f32, tag="px", name="px")
                for k in range(KT):
                    nc.tensor.matmul(
                        acc[:], wq_sb[:, k * DG + m * P:k * DG + (m + 1) * P],
                        xc[:, k, :], start=(k == 0), stop=(k == KT - 1))
                nc.vector.tensor_add(
                    qT[m][:, jj * CH:(jj + 1) * CH], acc[:],
                    bqk_sb[:, m:m + 1].broadcast_to([P, CH]))
            return fn

        # ---- attention: global software-pipelined iteration stream -----
        for j in range(NJ):
            jc = slice(j * CH, (j + 1) * CH)
            if j + 1 < NJ:
                xq_next = load_xchunk(xqT_d, j + 1)
                xq_stage = xq_next

            for m in range(2):
                # q-proj of j+1 staged at m=1 so its input DMA has landed
                if m == 1 and j + 1 < NJ:
                    for mm_ in range(2):
                        sprinkle.append(emit_qproj_unit(xq_stage, mm_, j + 1))
                # psum accumulators for this head pair (ctx 0-63, den 64)
                accs = [ps_acc.tile([P, CH], f32, tag="acc", name=f"acc{i}")
                        for i in range(2)]
                avs = [accs[0][0:VBLK, :], accs[1][0:VBLK, :]]

                for t in range(STILES):
                    # both heads' scores for sk-tile t in ONE psum tile so
                    # the pool's 2 bufs give true double-buffering, and one
                    # exp instruction covers both heads
                    sps = ps_s.tile([P, 2 * CH], f32, tag="s", name="s")
                    for hh in range(2):
                        lo, hi = hh * HD, (hh + 1) * HD
                        nc.tensor.matmul(
                            sps[:, hh * CH:(hh + 1) * CH],
                            kT[m][lo:hi, t * P:(t + 1) * P],
                            qT[m][lo:hi, jc], start=True, stop=True)
                    e = epool.tile([P, 2 * CH], MM_DT, tag="e", name="e")
                    nc.scalar.activation(e[:], sps[:], EXP, scale=SCALE)

                    def emit_pv(m=m, t=t, e=e, avs=avs):
                        for hh in range(2):
                            nc.tensor.matmul(
                                avs[hh], v_view[:, 2 * m + hh, t, :],
                                e[:, hh * CH:(hh + 1) * CH],
                                start=(t == 0), stop=(t == STILES - 1))
                        return False
                    pvq.append(emit_pv)
                    pop_pv()
                    pop_sprinkle()

                def emit_norm(m=m, j=j, jc=jc, accs=accs):
                    # normalize ctx by the softmax denominators (psum row 64
                    # of each acc).  DVE reciprocal cost scales with per-lane
                    # free size, so reshape the two [1,512] rows to [128,8]
                    # (sbuf->sbuf DMA crosses partitions), recip there, then
                    # bounce through DRAM for a stride-0 partition broadcast
                    # back to [64,512].  The whole chain runs on DVE/DMA and
                    # is deferred, so the PE never waits on it.
                    caccs = [small.tile([VBLK, CH], f32, tag="cacc",
                                        name="cacc") for _ in range(2)]
                    for hh in range(2):
                        nc.vector.tensor_copy(caccs[hh][:], accs[hh][:VBLK, :])
                    nq = CH // P
                    rq = small.tile([P, 2 * nq], f32, tag="rq", name="rq")
                    for hh in range(2):
                        nc.gpsimd.dma_start(
                            rq[:, hh * nq:(hh + 1) * nq],
                            caccs[hh][HD:HD + 1, :].rearrange(
                                "o (p c) -> o p c", p=P))
                    rr = small.tile([P, 2 * nq], f32, tag="rr", name="rr")
                    nc.vector.reciprocal(rr[:], rq[:])
                    drb = drbp.tile([2, CH], f32, tag="drb", name="drb")
                    for hh in range(2):
                        nc.gpsimd.dma_start(
                            drb[hh:hh + 1, :].rearrange(
                                "o (p c) -> (o p) c", p=P),
                            rr[:, hh * nq:(hh + 1) * nq])
                    bbs = []
                    for hh in range(2):
                        bb = bpool.tile([HD, CH], f32, tag="bb", name="bb")
                        nc.gpsimd.dma_start(
                            bb[:], drb[hh:hh + 1, :].broadcast_to([HD, CH]))
                        bbs.append(bb)
                    nc.vector.tensor_mul(ctxT[m][0:HD, jc],
                                         caccs[0][0:HD, :], bbs[0][:])
                    tmp = small.tile([HD, CH], MM_DT, tag="tmp", name="tmp")
                    nc.vector.tensor_mul(tmp[:], caccs[1][0:HD, :], bbs[1][:])
                    nc.gpsimd.dma_start(ctxT[m][HD:P, jc], tmp[:])
                    if m == 1:
                        # ctxT for chunk j complete: its out-proj may now be
                        # scheduled (pops during the next chunk's loop)
                        for si in range(CH // P):
                            for nh in range(2):
                                sprinkle.append(emit_outproj_unit(j, si, nh))
                    return True
                pvq.append(emit_norm)

        # ---- drain: remaining PV/normalize + final chunk out-proj ------
        while pvq:
            pvq.popleft()()
            pop_sprinkle()
        while sprinkle:
            sprinkle.popleft()()


_NC_CACHE = {}


def _get_program():
    if "nc" not in _NC_CACHE:
        _NC_CACHE["nc"] = _build_program()
    return _NC_CACHE["nc"]


def _make_in_maps(inputs):
    query = np.asarray(inputs["query"], dtype=np.float32)
    key = np.asarray(inputs["key"], dtype=np.float32)
    value = np.asarray(inputs["value"], dtype=np.float32)
    Wq = np.asarray(inputs["Wq"], dtype=np.float32)
    Wk = np.asarray(inputs["Wk"], dtype=np.float32)
    Wv = np.asarray(inputs["Wv"], dtype=np.float32)
    Wo = np.asarray(inputs["Wo"], dtype=np.float32)
    bq = np.asarray(inputs["bq"], dtype=np.float32)
    bk = np.asarray(inputs["bk"], dtype=np.float32)

    def xlayout(x):
        # [S, D] -> [P, NJ*KT*CH]: per partition, chunk-major with 8KB
        # contiguous runs (see dram_tensor comment in _build_program)
        a = x.T.reshape(KT, P, NJ, CH).transpose(1, 2, 0, 3)
        return np.ascontiguousarray(a).reshape(P, NJ * KT * CH).astype(NP_MM)

    def wlayout(w):
        # [D, M] -> [P, KT*M]: per-partition contiguous
        m = w.shape[1]
        a = w.reshape(KT, P, m).transpose(1, 0, 2)
        return np.ascontiguousarray(a).reshape(P, KT * m).astype(NP_MM)

    xT = {}
    for b in range(B):
        xT[("q", b)] = xlayout(query[b])
        xT[("k", b)] = xlayout(key[b])
        xT[("v", b)] = xlayout(value[b])

    in_maps = []
    for c in range(NCORE):
        b, g = divmod(c, G)
        cols = slice(g * DG, (g + 1) * DG)
        in_maps.append({
            "xqT": xT[("q", b)],
            "xkT": xT[("k", b)],
            "xvT": xT[("v", b)],
            "wq": wlayout(Wq[:, cols]),
            "wk": wlayout(Wk[:, cols]),
            "wv": wlayout(Wv[:, cols]),
            "wo": Wo[cols, :].reshape(2, P, D).transpose(1, 0, 2).reshape(
                P, 2 * D).astype(NP_MM),
            "bqk": np.ascontiguousarray(np.stack([bq[cols], bk[cols]])),
        })
    return in_maps


def kernel(query, key, value, Wq, bq, Wk, bk, Wv, bv, Wo, bo):
    bv = np.asarray(bv, dtype=np.float32)
    bo = np.asarray(bo, dtype=np.float32)
    Wo = np.asarray(Wo, dtype=np.float32)

    nc = _get_program()
    in_maps = _make_in_maps({
        "query": query, "key": key, "value": value, "Wq": Wq, "Wk": Wk,
        "Wv": Wv, "Wo": Wo, "bq": bq, "bk": bk,
    })

    res = run_bass_kernel_spmd(nc, in_maps, list(range(NCORE)))

    # unshard: sum the 4 head-group partials per batch; add the linear bias
    # correction (bv and bo commute through attention/out_proj).
    corr = bv @ Wo + bo
    out = np.empty((B, S, D), dtype=np.float32)
    for b in range(B):
        acc = res.results[4 * b]["out"].copy()
        for g in range(1, G):
            acc += res.results[4 * b + g]["out"]
        out[b] = acc + corr
    return out


# revision 30
# speedup vs baseline: 1.1984x; 1.0195x over previous
"""Multi-head attention (B=2, S=2048, D=1024, H=16) on 8 Trainium2 NeuronCores.

Sharding (per the batch+head hint): core c handles batch b=c//4 and head-group
g=c%4 (4 heads, i.e. a 256-column slice of the QKV projections and a 256-row
slice of Wo).  Each core computes q^T/k^T/v projections for its head group,
flash-style attention in transposed score space (scores^T = k^T-tile.T @ q^T,
softmax denominator via a ones-augmented V column in the PV matmul), and its
out-projection partial  ctx_g @ Wo[256g:256(g+1), :].

The out_proj reduction over the 4 head-group cores of each batch is done on
the host (device collectives on this stack cost ~145us for 8MB - far more
than the arithmetic they replace - so the partial-sum gather IS the unshard
step).  Biases: bq/bk are applied on device (they feed the softmax
nonlinearly); bv/bo commute through attention/out_proj linearly and are folded
into a single host-side correction vector  c = bv @ Wo + bo.

Performance structure (v2): the PE array is the binding engine, so emission
is software-pipelined to keep it saturated:
 - all matmul operands are bf16 (1 col/cycle streaming; LDWEIGHTS pipelines
   behind the previous matmul, so back-to-back cadence is ~N/2.4GHz).
 - PV matmuls lag the scores/exp stream by a few iterations so the PE never
   waits on the Activation engine's exp.
 - out-projection of chunk j-1 and q-projection of chunk j+1 are sprinkled
   into chunk j's attention loop to fill residual exp-wait gaps.
 - softmax normalization is fully on-chip: for odd heads the V block is
   stored [ones|v] and the PV output lands at psum partitions 63..127, so
   ctx rows 64..127 are written in place (no partition-shift DMA).  The
   denominator reciprocal row is broadcast across partitions with a rank-1
   matmul (ones ⊗ recip) instead of a DRAM round trip.
 - PSUM->SBUF copies ride the otherwise-idle GpSimd engine.
"""

import numpy as np
import ml_dtypes

import concourse.bass as bass
import concourse.mybir as mybir
import concourse.tile as tile
from concourse import bacc
from concourse.bass_utils import run_bass_kernel_spmd

B, S, D, H = 2, 2048, 1024, 16
HD = D // H          # 64 head dim
NCORE = 8
G = NCORE // B       # 4 head-groups per batch
HG = H // G          # 4 heads per group
DG = D // G          # 256 projection columns per group
P = 128              # partitions
KT = D // P          # 8 contraction tiles for projections
CH = 512             # s-chunk (projection rhs width & attention sq chunk)
NJ = S // CH         # 4 chunks
STILES = S // P      # 16 sk tiles
TB = 2               # scores t-batch per exp op (psum: [128, TB*512] = 2 banks)
VBLK = HD + 1        # v block: 64 v cols + 1 ones col (softmax denominator)
LAG = 3              # PV emission lags scores/exp by this many tb-iterations

f32 = mybir.dt.float32
bf16 = mybir.dt.bfloat16
MM_DT = mybir.dt.bfloat16
NP_MM = np.float32 if MM_DT == mybir.dt.float32r else ml_dtypes.bfloat16
EXP = mybir.ActivationFunctionType.Exp
SCALE = 1.0 / np.sqrt(np.float32(HD))


def _build_program():
    nc = bacc.Bacc("TRN2", target_bir_lowering=False, debug=False,
                   num_devices=NCORE)

    # inputs/weights are pre-laid-out on the host so every DMA reads
    # contiguous 4-8KB runs per partition (strided 1KB-run descriptors only
    # reach ~100GB/s on these queues)
    xqT_d = nc.dram_tensor("xqT", [P, NJ * KT * CH], MM_DT, kind="ExternalInput")
    xkT_d = nc.dram_tensor("xkT", [P, NJ * KT * CH], MM_DT, kind="ExternalInput")
    xvT_d = nc.dram_tensor("xvT", [P, NJ * KT * CH], MM_DT, kind="ExternalInput")
    wq_d = nc.dram_tensor("wq", [P, KT * DG], MM_DT, kind="ExternalInput")
    wk_d = nc.dram_tensor("wk", [P, KT * DG], MM_DT, kind="ExternalInput")
    wv_d = nc.dram_tensor("wv", [P, KT * DG], MM_DT, kind="ExternalInput")
    wo_d = nc.dram_tensor("wo", [P, 2 * D], MM_DT, kind="ExternalInput")
    bq_d = nc.dram_tensor("bqk", [2, DG], f32, kind="ExternalInput")
    out_d = nc.dram_tensor("out", [S, D], f32, kind="ExternalOutput")

    with tile.TileContext(nc) as tc:
        _emit(nc, tc, xqT_d, xkT_d, xvT_d, wq_d, wk_d, wv_d, wo_d, bq_d, out_d)
    nc.compile()
    return nc


def _emit(nc, tc, xqT_d, xkT_d, xvT_d, wq_d, wk_d, wv_d, wo_d, bq_d, out_d):
    from collections import deque
    from contextlib import ExitStack
    ctx = ExitStack()
    with ctx:
        consts = ctx.enter_context(tc.tile_pool(name="consts", bufs=1))
        persist = ctx.enter_context(tc.tile_pool(name="persist", bufs=1))
        xpool = ctx.enter_context(tc.tile_pool(name="xchunk", bufs=6))
        epool = ctx.enter_context(tc.tile_pool(name="exps", bufs=10))
        small = ctx.enter_context(tc.tile_pool(name="small", bufs=8))
        bpool = ctx.enter_context(tc.tile_pool(name="bcast", bufs=4))
        opool = ctx.enter_context(tc.tile_pool(name="ostage", bufs=3))
        drbp = ctx.enter_context(tc.tile_pool(name="drb", bufs=4, space="DRAM"))
        ps_s = ctx.enter_context(tc.tile_pool(name="ps_s", bufs=2, space="PSUM"))
        ps_acc = ctx.enter_context(tc.tile_pool(name="ps_acc", bufs=2, space="PSUM"))
        ps_x = ctx.enter_context(tc.tile_pool(name="ps_x", bufs=2, space="PSUM"))

        def load_xchunk(x_d, j, eng=None):
            # spread input streams across DGE queues so transfers overlap
            t = xpool.tile([P, KT * CH], MM_DT, tag="xchunk", name="xchunk")
            (eng or nc.sync).dma_start(
                out=t[:],
                in_=x_d.rearrange("p (j x) -> p j x", j=NJ)[:, j])
            return t.rearrange("p (kt s) -> p kt s", kt=KT)

        # ---- constants: wk first (k-projection starts the kernel) -----
        wk_sb = consts.tile([P, KT * DG], MM_DT, tag="wk")
        nc.sync.dma_start(out=wk_sb[:], in_=wk_d.ap())
        xc0 = load_xchunk(xkT_d, 0)

        wq_sb = consts.tile([P, KT * DG], MM_DT, tag="wq")
        wv_sb = consts.tile([P, KT * DG], MM_DT, tag="wv")
        nc.scalar.dma_start(out=wq_sb[:], in_=wq_d.ap())
        nc.gpsimd.dma_start(out=wv_sb[:], in_=wv_d.ap())
        wo_sb = consts.tile([P, 2 * D], MM_DT, tag="wo")  # 2 k-tiles [128, D]
        nc.sync.dma_start(out=wo_sb[:], in_=wo_d.ap())
        bqk_sb = consts.tile([P, 4], f32, tag="bqk")  # [bq|bk] x m-half
        nc.sync.dma_start(out=bqk_sb.rearrange("p (i mh) -> p i mh", i=2),
                          in_=bq_d.rearrange("i (mh p) -> p i mh", p=P))
        # persistent activations
        qT = [persist.tile([P, S], MM_DT, tag=f"qT{m}", name=f"qT{m}")
              for m in range(2)]
        kT = [persist.tile([P, S], MM_DT, tag=f"kT{m}", name=f"kT{m}")
              for m in range(2)]
        v_sb = persist.tile([P, HG * STILES * VBLK], MM_DT, tag="v")
        ctxT = [persist.tile([P, S], MM_DT, tag=f"ctxT{m}", name=f"ctxT{m}")
                for m in range(2)]
        v_view = v_sb.rearrange("p (h t c) -> p h t c", h=HG, t=STILES)

        # ones columns for the softmax-denominator rows of the PV matmuls
        nc.vector.memset(v_view[:, :, :, HD], 1.0)

        def proj_qk(xc, w_sb, dst, bias_i, j):
            # dst[m][dq, j*CH:+CH] = (W[:, m-half].T @ x^T-chunk) + bias
            for m in range(2):
                acc = ps_x.tile([P, CH], f32, tag="px", name="px")
                for k in range(KT):
                    nc.tensor.matmul(
                        acc[:], w_sb[:, k * DG + m * P:k * DG + (m + 1) * P],
                        xc[:, k, :], start=(k == 0), stop=(k == KT - 1))
                nc.vector.tensor_add(
                    dst[m][:, j * CH:(j + 1) * CH], acc[:],
                    bqk_sb[:, 2 * bias_i + m:2 * bias_i + m + 1].broadcast_to(
                        [P, CH]))

        def proj_v(xc, j):
            # v rows j*CH..: 4 s-subtiles of 128; heads land in v_view blocks
            for si in range(CH // P):
                st = j * (CH // P) + si
                acc = ps_x.tile([P, DG], f32, tag="px", name="px")
                for k in range(KT):
                    nc.tensor.matmul(
                        acc[:], xc[:, k, si * P:(si + 1) * P],
                        wv_sb[:, k * DG:(k + 1) * DG],
                        start=(k == 0), stop=(k == KT - 1))
                nc.scalar.activation(
                    v_view[:, :, st, 0:HD],
                    acc[:].rearrange("p (h c) -> p h c", h=HG),
                    mybir.ActivationFunctionType.Copy)

        # ---- head phase: k (full S), q chunk 0, v (full S) -------------
        # xq/xv ride the Activation/GpSimd DGE queues so their transfers
        # overlap the xk stream on the SP queue.
        xq0 = load_xchunk(xqT_d, 0, eng=nc.scalar)
        xvc = [load_xchunk(xvT_d, j, eng=nc.gpsimd) for j in range(2)]
        proj_qk(xc0, wk_sb, kT, 1, 0)
        for j in range(1, NJ):
            xc = load_xchunk(xkT_d, j)
            proj_qk(xc, wk_sb, kT, 1, j)
        proj_qk(xq0, wq_sb, qT, 0, 0)
        for j in range(NJ):
            proj_v(xvc[j], j)
            if j + 2 < NJ:
                xvc.append(load_xchunk(xvT_d, j + 2, eng=nc.gpsimd))

        # ---- deferred-work queues --------------------------------------
        # pvq: ordered PV/normalize closures (softmax-side pipeline, popped
        #      1 per tb-iteration, lagging production by LAG iterations).
        # sprinkle: independent PE work (out-proj of j-1, q-proj of j+1)
        #      popped up to 2 units per iteration to fill exp-wait gaps.
        pvq = deque()
        sprinkle = deque()

        def pop_pv():
            if len(pvq) > LAG:
                fn = pvq.popleft()
                again = fn()
                if again and len(pvq) > LAG:  # normalize is cheap: pop 2
                    pvq.popleft()()

        def pop_sprinkle(n=2):
            for _ in range(min(n, len(sprinkle))):
                sprinkle.popleft()()

        def emit_outproj_unit(jj, si, nh, tail=False):
            def fn():
                st = jj * (CH // P) + si
                if tail:
                    po = ps_s.tile([P, 2 * CH], f32, tag="s", name="s")[:, 0:CH]
                else:
                    po = ps_x.tile([P, CH], f32, tag="px", name="px")
                for m in range(2):
                    nc.tensor.matmul(
                        po[:], ctxT[m][:, st * P:(st + 1) * P],
                        wo_sb[:, m * D + nh * CH:m * D + (nh + 1) * CH],
                        start=(m == 0), stop=(m == 1))
                ostage = opool.tile([P, CH], f32, tag="ostage", name="ostage")
                nc.vector.tensor_copy(ostage[:], po[:])
                (nc.scalar if tail else nc.sync).dma_start(
                    out_d[st * P:(st + 1) * P, nh * CH:(nh + 1) * CH],
                    ostage[:])
            return fn

        def emit_qproj_unit(xc, m, jj):
            def fn():
                acc = ps_x.tile([P, CH], f32, tag="px", name="px")
                for k in range(KT):
                    nc.tensor.matmul(
                        acc[:], wq_sb[:, k * DG + m * P:k * DG + (m + 1) * P],
                        xc[:, k, :], start=(k == 0), stop=(k == KT - 1))
                nc.vector.tensor_add(
                    qT[m][:, jj * CH:(jj + 1) * CH], acc[:],
                    bqk_sb[:, m:m + 1].broadcast_to([P, CH]))
            return fn

        # ---- attention: global software-pipelined iteration stream -----
        for j in range(NJ):
            jc = slice(j * CH, (j + 1) * CH)
            if j + 1 < NJ:
                xq_next = load_xchunk(xqT_d, j + 1)
                xq_stage = xq_next

            for m in range(2):
                # q-proj of j+1 staged at m=1 so its input DMA has landed
                if m == 1 and j + 1 < NJ:
                    for mm_ in range(2):
                        sprinkle.append(emit_qproj_unit(xq_stage, mm_, j + 1))
                # psum accumulators for this head pair (ctx 0-63, den 64)
                accs = [ps_acc.tile([P, CH], f32, tag="acc", name=f"acc{i}")
                        for i in range(2)]
                avs = [accs[0][0:VBLK, :], accs[1][0:VBLK, :]]

                for t in range(STILES):
                    # both heads' scores for sk-tile t in ONE psum tile so
                    # the pool's 2 bufs give true double-buffering, and one
                    # exp instruction covers both heads
                    sps = ps_s.tile([P, 2 * CH], f32, tag="s", name="s")
                    for hh in range(2):
                        lo, hi = hh * HD, (hh + 1) * HD
                        nc.tensor.matmul(
                            sps[:, hh * CH:(hh + 1) * CH],
                            kT[m][lo:hi, t * P:(t + 1) * P],
                            qT[m][lo:hi, jc], start=True, stop=True)
                    e = epool.tile([P, 2 * CH], MM_DT, tag="e", name="e")
                    nc.scalar.activation(e[:], sps[:], EXP, scale=SCALE)

                    def emit_pv(m=m, t=t, e=e, avs=avs):
                        for hh in range(2):
                            nc.tensor.matmul(
                                avs[hh], v_view[:, 2 * m + hh, t, :],
                                e[:, hh * CH:(hh + 1) * CH],
                                start=(t == 0), stop=(t == STILES - 1))
                        return False
                    pvq.append(emit_pv)
                    pop_pv()
                    pop_sprinkle()

                def emit_norm(m=m, j=j, jc=jc, accs=accs):
                    # the very last norm has nothing to hide behind: use the
                    # idle scalar hwdge queue instead of the software DGE
                    last = (j == NJ - 1 and m == 1)
                    dmae = nc.scalar if last else nc.gpsimd
                    # normalize ctx by the softmax denominators (psum row 64
                    # of each acc).  DVE reciprocal cost scales with per-lane
                    # free size, so reshape the two [1,512] rows to [128,8]
                    # (sbuf->sbuf DMA crosses partitions), recip there, then
                    # bounce through DRAM for a stride-0 partition broadcast
                    # back to [64,512].  The whole chain runs on DVE/DMA and
                    # is deferred, so the PE never waits on it.
                    caccs = [small.tile([VBLK, CH], f32, tag="cacc",
                                        name="cacc") for _ in range(2)]
                    for hh in range(2):
                        nc.vector.tensor_copy(caccs[hh][:], accs[hh][:VBLK, :])
                    nq = CH // P
                    rq = small.tile([P, 2 * nq], f32, tag="rq", name="rq")
                    for hh in range(2):
                        dmae.dma_start(
                            rq[:, hh * nq:(hh + 1) * nq],
                            caccs[hh][HD:HD + 1, :].rearrange(
                                "o (p c) -> o p c", p=P))
                    rr = small.tile([P, 2 * nq], f32, tag="rr", name="rr")
                    nc.vector.reciprocal(rr[:], rq[:])
                    drb = drbp.tile([2, CH], f32, tag="drb", name="drb")
                    for hh in range(2):
                        dmae.dma_start(
                            drb[hh:hh + 1, :].rearrange(
                                "o (p c) -> (o p) c", p=P),
                            rr[:, hh * nq:(hh + 1) * nq])
                    bbs = []
                    for hh in range(2):
                        bb = bpool.tile([HD, CH], f32, tag="bb", name="bb")
                        dmae.dma_start(
                            bb[:], drb[hh:hh + 1, :].broadcast_to([HD, CH]))
                        bbs.append(bb)
                    nc.vector.tensor_mul(ctxT[m][0:HD, jc],
                                         caccs[0][0:HD, :], bbs[0][:])
                    tmp = small.tile([HD, CH], MM_DT, tag="tmp", name="tmp")
                    nc.vector.tensor_mul(tmp[:], caccs[1][0:HD, :], bbs[1][:])
                    nc.gpsimd.dma_start(ctxT[m][HD:P, jc], tmp[:])
                    if m == 1:
                        # ctxT for chunk j complete: its out-proj may now be
                        # scheduled (pops during the next chunk's loop)
                        for si in range(CH // P):
                            for nh in range(2):
                                sprinkle.append(emit_outproj_unit(
                                    j, si, nh, tail=(j == NJ - 1)))
                    return True
                pvq.append(emit_norm)

        # ---- drain: remaining PV/normalize + final chunk out-proj ------
        while pvq:
            pvq.popleft()()
            pop_sprinkle()
        while sprinkle:
            sprinkle.popleft()()


_NC_CACHE = {}


def _get_program():
    if "nc" not in _NC_CACHE:
        _NC_CACHE["nc"] = _build_program()
    return _NC_CACHE["nc"]


def _make_in_maps(inputs):
    query = np.asarray(inputs["query"], dtype=np.float32)
    key = np.asarray(inputs["key"], dtype=np.float32)
    value = np.asarray(inputs["value"], dtype=np.float32)
    Wq = np.asarray(inputs["Wq"], dtype=np.float32)
    Wk = np.asarray(inputs["Wk"], dtype=np.float32)
    Wv = np.asarray(inputs["Wv"], dtype=np.float32)
    Wo = np.asarray(inputs["Wo"], dtype=np.float32)
    bq = np.asarray(inputs["bq"], dtype=np.float32)
    bk = np.asarray(inputs["bk"], dtype=np.float32)

    def xlayout(x):
        # [S, D] -> [P, NJ*KT*CH]: per partition, chunk-major with 8KB
        # contiguous runs (see dram_tensor comment in _build_program)
        a = x.T.reshape(KT, P, NJ, CH).transpose(1, 2, 0, 3)
        return np.ascontiguousarray(a).reshape(P, NJ * KT * CH).astype(NP_MM)

    def wlayout(w):
        # [D, M] -> [P, KT*M]: per-partition contiguous
        m = w.shape[1]
        a = w.reshape(KT, P, m).transpose(1, 0, 2)
        return np.ascontiguousarray(a).reshape(P, KT * m).astype(NP_MM)

    xT = {}
    for b in range(B):
        xT[("q", b)] = xlayout(query[b])
        xT[("k", b)] = xlayout(key[b])
        xT[("v", b)] = xlayout(value[b])

    in_maps = []
    for c in range(NCORE):
        b, g = divmod(c, G)
        cols = slice(g * DG, (g + 1) * DG)
        in_maps.append({
            "xqT": xT[("q", b)],
            "xkT": xT[("k", b)],
            "xvT": xT[("v", b)],
            "wq": wlayout(Wq[:, cols]),
            "wk": wlayout(Wk[:, cols]),
            "wv": wlayout(Wv[:, cols]),
            "wo": Wo[cols, :].reshape(2, P, D).transpose(1, 0, 2).reshape(
                P, 2 * D).astype(NP_MM),
            "bqk": np.ascontiguousarray(np.stack([bq[cols], bk[cols]])),
        })
    return in_maps


def kernel(query, key, value, Wq, bq, Wk, bk, Wv, bv, Wo, bo):
    bv = np.asarray(bv, dtype=np.float32)
    bo = np.asarray(bo, dtype=np.float32)
    Wo = np.asarray(Wo, dtype=np.float32)

    nc = _get_program()
    in_maps = _make_in_maps({
        "query": query, "key": key, "value": value, "Wq": Wq, "Wk": Wk,
        "Wv": Wv, "Wo": Wo, "bq": bq, "bk": bk,
    })

    res = run_bass_kernel_spmd(nc, in_maps, list(range(NCORE)))

    # unshard: sum the 4 head-group partials per batch; add the linear bias
    # correction (bv and bo commute through attention/out_proj).
    corr = bv @ Wo + bo
    out = np.empty((B, S, D), dtype=np.float32)
    for b in range(B):
        acc = res.results[4 * b]["out"].copy()
        for g in range(1, G):
            acc += res.results[4 * b + g]["out"]
        out[b] = acc + corr
    return out
